# revision 1
# baseline (speedup 1.0000x reference)
"""Trainium2 Bass kernel for a 3-layer edge-featured GAT over 256 dense 84-node graphs.

Contract: kernel(**inputs) takes the FULL unsharded inputs and returns the FULL
[256, 1] float32 output. Data parallel over graphs: 32 graphs/core on 8 cores.

v2 design (scaled-carry, per-graph projections, minimal DMA count):
  - Carry S = [features(64) | den] kept UNNORMALIZED (scaled per node by the
    previous layer's softmax denominator). Projections use S_g as the PE
    stationary per graph with the combined weight CW moving, so the projected
    [W h | den | adst | asrc] comes out NODE-major directly (no feature-major
    intermediate). One reciprocal + bcast-multiply per PSUM bank normalizes by
    the den column, restoring true attention scalars, turning den into the
    ones column for aggregation, and applying the layer bias (folded
    host-side into CW against the den row).
  - Attention rows: per-graph [84,2] PE transposes of the normalized att cols
    -> contiguous [2, NB] rows -> 2-3 small DMAs fill the a_src stationary
    rows and the a_dst moving row.
  - Logits: host bakes E planes + periodic 4-graph masks + an adst placeholder
    row into one [89, NB] tensor per layer (1 DMA each). Stationary [89,84]
    per chunk = identity + runtime a_src rows + ones.
  - ex = exp(lrelu(pl)): per-chunk lrelu (Scalar ACT / DVE stt alternating)
    into SBUF + 2 half-width Scalar exps.
  - Aggregation: stationary = hnode [W h|1] cols, moving = ex_g; output is
    feature-major [agg(64) | den] and the PSUM->SBUF relu copy IS the next
    carry (relu(den)=den since den>0).
  - Readout: stationary ex_g, moving interleaved [v|1]; per-dst ratio, then
    one f32 matmul against a ones column pools each graph; relu+bias; 1 DMA.
"""

import sys

for _p in ("/opt/trn_rl_repo",):
    if _p not in sys.path:
        sys.path.append(_p)

import numpy as np

from contextlib import ExitStack

from concourse import bacc, bass, mybir, tile
from concourse.bass_types import AP
from concourse.bass_utils import run_bass_kernel_spmd

F32 = mybir.dt.float32
F16 = mybir.dt.float16
AF = mybir.ActivationFunctionType
ALU = mybir.AluOpType

NPG = 84            # nodes per graph
B = 256             # graphs
HID = 64
DEPTH = 3
NEG = 0.2
NC_CORES = 8
GPC = B // NC_CORES     # 32 graphs per core
NB = GPC * NPG          # 2688 nodes per core
CH = 4                  # graphs per logits chunk / proj bank / agg bank
NCH = GPC // CH         # 8
CHW = CH * NPG          # 336 cols per chunk


def _host_preprocess(inputs):
    x = np.ascontiguousarray(np.asarray(inputs['x'], np.float32))
    ei = np.asarray(inputs['edge_index'])
    ea = np.asarray(inputs['edge_attr'], np.float32)
    W0 = np.asarray(inputs['W0'], np.float32)
    Ws = np.asarray(inputs['Ws'], np.float32)
    asl = np.asarray(inputs['att_src_all'], np.float32)
    adl = np.asarray(inputs['att_dst_all'], np.float32)
    Wel = np.asarray(inputs['W_edge_all'], np.float32)
    ael = np.asarray(inputs['att_edge_all'], np.float32)
    bl = np.asarray(inputs['bias_all'], np.float32)
    linW = np.asarray(inputs['lin_W'], np.float32)
    linb = np.asarray(inputs['lin_b'], np.float32)

    src, dst = np.asarray(ei[0]), np.asarray(ei[1])
    g = src // NPG
    assert np.all(dst // NPG == g), "edges cross graph boundaries"
    sl, dl = src % NPG, dst % NPG

    dense = np.zeros((B, NPG, NPG, 2), np.float32)
    dense[g, sl, dl] = ea
    cnt = np.zeros((B, NPG), np.float32)
    np.add.at(cnt, (g, dl), 1.0)
    colsum = dense.sum(axis=1)
    loop_attr = colsum / np.maximum(cnt, 1.0)[..., None]
    di = np.arange(NPG)
    dense[:, di, di, :] = loop_attr

    Es = [np.ascontiguousarray(dense @ (Wel[l] @ ael[l]), np.float32)
          for l in range(DEPTH)]   # [B, s, d]

    W_all = [W0, Ws[0], Ws[1]]
    # CW cols = [W(0:64) | asrc(64) | den(65) | adst(66)]. Carries for layers
    # 1,2 are [feat(0:64) | junk(64) | den(65) | junk(66)] so middle CWs have
    # 67 input rows with zeroed junk rows.
    CW = []
    for l in range(2):
        K = W_all[l].shape[0]
        KR = 2 if l == 0 else 67
        A = np.zeros((KR, 67), np.float32)
        dr = 1 if l == 0 else 65   # den row of the incoming carry
        A[0:K, 0:64] = W_all[l]
        A[0:K, 64] = W_all[l] @ asl[l]
        A[dr, 65] = 1.0
        A[0:K, 66] = W_all[l] @ adl[l]
        A[dr, 0:64] += bl[l]       # bias fold (scaled by den via the den row)
        CW.append(np.ascontiguousarray(A, np.float16))
    # CW2: [67, 4] cols = [v | asrc | den | adst]
    C2 = np.zeros((67, 4), np.float32)
    C2[0:64, 0] = (W_all[2] @ linW)[:, 0]
    C2[0:64, 1] = W_all[2] @ asl[2]
    C2[65, 2] = 1.0
    C2[0:64, 3] = W_all[2] @ adl[2]
    C2 = np.ascontiguousarray(C2, np.float16)

    tail_bias = float(NPG * float(bl[2] @ linW[:, 0]) + float(linb[0]))

    x_aug = np.ones((2, B * NPG), np.float16)
    x_aug[0] = x[:, 0].astype(np.float16)

    return dict(x_aug=x_aug, Es=Es, CW=CW, C2=C2, tail_bias=tail_bias)


def _core_inputs(pre, c):
    m = {
        'xh': np.ascontiguousarray(pre['x_aug'][:, c * NB:(c + 1) * NB]),
        'cw0': pre['CW'][0], 'cw1': pre['CW'][1], 'cw2': pre['C2'],
    }
    for l in range(DEPTH):
        m[f'EH{l}'] = np.ascontiguousarray(
            np.transpose(pre['Es'][l][c * GPC:(c + 1) * GPC], (1, 0, 2))
            .reshape(NPG, NB).astype(np.float16))
    return m


def _bcast_inner(ap, n):
    return AP(ap.tensor, ap.offset, list(ap.ap) + [[0, n]])


def _build_program(tail_bias):
    nc = bacc.Bacc("TRN2", target_bir_lowering=False, debug=False)

    xh_d = nc.dram_tensor("xh", [2, NB], F16, kind="ExternalInput").ap()
    EH_d = [nc.dram_tensor(f"EH{l}", [NPG, NB], F16, kind="ExternalInput").ap()
            for l in range(DEPTH)]
    cw_d = [nc.dram_tensor("cw0", [2, 67], F16, kind="ExternalInput").ap(),
            nc.dram_tensor("cw1", [67, 67], F16, kind="ExternalInput").ap(),
            nc.dram_tensor("cw2", [67, 4], F16, kind="ExternalInput").ap()]
    out_d = nc.dram_tensor("out", [GPC], F32, kind="ExternalOutput").ap()

    with tile.TileContext(nc) as tc, ExitStack() as ctx:
        cpool = ctx.enter_context(tc.tile_pool(name="const", bufs=1))
        spool = ctx.enter_context(tc.tile_pool(name="carry", bufs=1))
        hpool = ctx.enter_context(tc.tile_pool(name="hnode", bufs=2))
        expool = ctx.enter_context(tc.tile_pool(name="ex", bufs=2))
        lrpool = ctx.enter_context(tc.tile_pool(name="lr", bufs=2))
        atpool = ctx.enter_context(tc.tile_pool(name="att", bufs=2))
        lhpool = ctx.enter_context(tc.tile_pool(name="lhs", bufs=1))
        smpool = ctx.enter_context(tc.tile_pool(name="small", bufs=3))

        ps_hn = ctx.enter_context(tc.tile_pool(name="pshn", bufs=2, space="PSUM"))
        ps_pl = ctx.enter_context(tc.tile_pool(name="pspl", bufs=2, space="PSUM"))
        ps_ag = ctx.enter_context(tc.tile_pool(name="psag", bufs=2, space="PSUM"))
        ps_at = ctx.enter_context(tc.tile_pool(name="psat", bufs=2, space="PSUM"))

        # ---- constants / inputs (gpsimd first: x + cw0 gate the L0 proj) ----
        x_sb = spool.tile([2, NB], F16, tag="x")
        nc.sync.dma_start(x_sb[:], xh_d[:])
        cw_sb = []
        for l in range(DEPTH):
            t = cpool.tile(list(cw_d[l].shape), F16, tag=f"cw{l}",
                           name=f"cw{l}")
            cw_sb.append(t)
        nc.sync.dma_start(cw_sb[0][:], cw_d[0][:])

        # identity built on device (the packetized eye DMA is slower than
        # the whole E stream); ones row placed by a 1-packet SBUF->SBUF DMA.
        eye84 = cpool.tile([NPG, NPG], F16, tag="eye84")
        nc.gpsimd.memset(eye84[:], 1.0)
        nc.gpsimd.affine_select(eye84[:], eye84[:], [[1, NPG]],
                                ALU.is_equal, 0.0, base=0,
                                channel_multiplier=-1)
        # att staging: rows 0:3 = (asrc, ones, adst) from the transposes;
        # attR = (ones, adst) rebased to partition 0 for the rank-2 att
        # matmuls (adst arrives via a 1-packet partition-shift DMA per half).
        att_sb, attR_sb = [], []
        for i in range(2):
            t = lhpool.tile([3, NB], F16, tag=f"att{i}")
            att_sb.append(t)
            t2 = lhpool.tile([2, NB], F16, tag=f"attR{i}")
            attR_sb.append(t2)
            nc.vector.memset(t2[0:1, :], 1.0)

        # E loads split across parallel DMA queues (sync/scalar carry only E;
        # layer 0 adds a gpsimd share for earliest availability).
        bigE = [cpool.tile([NPG, NB], F16, tag=f"bigE{l}", name=f"bigE{l}")
                for l in range(DEPTH)]

        def load_E(l):
            for eng, r0, r1 in ((nc.sync, 0, 42), (nc.scalar, 42, 84)):
                eng.dma_start(bigE[l][r0:r1, :], EH_d[l][r0:r1, :])

        nc.gpsimd.dma_start(cw_sb[1][:], cw_d[1][:])
        nc.gpsimd.dma_start(cw_sb[2][:], cw_d[2][:])
        load_E(0)

        ones65 = smpool.tile([67, 2], F16, tag="ones65")
        nc.vector.memset(ones65[:], 1.0)

        S = x_sb

        def layer(l, S_in):
            ncw = 67 if l < 2 else 4
            # ---- projection: per-graph stationary S_g, moving CW ----
            if l < 2:
                hnode = hpool.tile([NPG, GPC * 67], F16, tag="hn")
            else:
                hnode = smpool.tile([NPG, GPC * 4], F16, tag="hn2")
            recipn = smpool.tile([NPG, GPC], F32, tag="recipn")
            dcol = 65 if l < 2 else 2
            for g0 in range(0, GPC, 7):
                g1 = min(g0 + 7, GPC)
                ng = g1 - g0
                ps = ps_hn.tile([NPG, 7 * ncw], F32, tag="ps")
                for j in range(ng):
                    nc.tensor.matmul(ps[:, j * ncw:(j + 1) * ncw],
                                     S_in[:, (g0 + j) * NPG:(g0 + j + 1) * NPG],
                                     cw_sb[l][:], start=True, stop=True)
                nc.vector.reciprocal(recipn[:, g0:g1],
                                     ps[:, dcol:ng * ncw:ncw])
                ps3 = ps[:, 0:ng * ncw].rearrange("p (g c) -> p g c", c=ncw)
                hn3 = (hnode[:, g0 * ncw:g1 * ncw]
                       .rearrange("p (g c) -> p g c", c=ncw))
                nc.vector.scalar_tensor_tensor(
                    hn3, ps3, 1.0, _bcast_inner(recipn[:, g0:g1], ncw),
                    ALU.mult, ALU.mult)
            # ---- attention rows: per-graph [84,3] transposes of the
            # (asrc, ones, adst) cols; adst rebased to attR row 1 by a
            # 1-packet partition-shift DMA per half ----
            acol = 64 if l < 2 else 1
            att = att_sb[l % 2]
            attR = attR_sb[l % 2]
            for b in range(4):
                pt = ps_at.tile([3, 2 * CHW], F16, tag="pt")
                for j in range(2 * CH):
                    g = 2 * CH * b + j
                    nc.tensor.transpose(
                        pt[:, j * NPG:(j + 1) * NPG],
                        hnode[:, g * ncw + acol:g * ncw + acol + 3],
                        eye84[:])
                bs = slice(b * 2 * CHW, (b + 1) * 2 * CHW)
                nc.scalar.copy(att[0:3, bs], pt[:])
                if b % 2 == 1:
                    hw = NB // 2
                    h0 = (b // 2) * hw
                    hs = slice(h0, h0 + hw)
                    nc.gpsimd.dma_start(attR[1:2, hs], att[2:3, hs])
            if l + 1 < DEPTH:
                load_E(l + 1)

            # ---- logits; ex = exp(prelu(pl, 0.2)) ----
            ex = expool.tile([NPG, NB], F16, tag="ex")
            lr = lrpool.tile([NPG, NB], F16, tag="lr")
            for half in range(4):
                for c in range(2 * half, 2 * half + 2):
                    cs = slice(c * CHW, (c + 1) * CHW)
                    pl = ps_pl.tile([NPG, CHW], F32, tag="pl")
                    # E first: start=True zeroes the whole PSUM zero-region,
                    # so the rank-2 att matmuls must accumulate afterwards.
                    nc.tensor.matmul(pl[:], eye84[:], bigE[l][:, cs],
                                     start=True, stop=False,
                                     skip_group_check=True)
                    for j in range(CH):
                        g = CH * c + j
                        gs = slice(g * NPG, (g + 1) * NPG)
                        nc.tensor.matmul(pl[:, j * NPG:(j + 1) * NPG],
                                         att[0:2, gs], attR[0:2, gs],
                                         start=False, stop=(j == CH - 1),
                                         skip_group_check=True)
                    nc.scalar.activation(lr[:, cs], pl[:], AF.Prelu,
                                         alpha=NEG)
                hs = slice(half * 2 * CHW, (half + 1) * 2 * CHW)
                nc.scalar.activation(ex[:, hs], lr[:, hs], AF.Exp)
            return hnode, ex

        # ---- layers 0,1 ----
        for l in range(2):
            hnode, ex = layer(l, S)
            S_next = spool.tile([67, NB], F16, tag=f"S{l + 1}")
            for b in range(NCH):
                pa = ps_ag.tile([67, CHW], F32, tag="pa")
                for j in range(CH):
                    g = CH * b + j
                    nc.tensor.matmul(pa[:, j * NPG:(j + 1) * NPG],
                                     hnode[:, g * 67:g * 67 + 67],
                                     ex[:, g * NPG:(g + 1) * NPG],
                                     start=True, stop=True)
                bs = slice(b * CHW, (b + 1) * CHW)
                pa3 = pa[:].rearrange("p (o c) -> p o c", o=1)
                sn3 = S_next[:, bs].rearrange("p (o c) -> p o c", o=1)
                nc.vector.scalar_tensor_tensor(
                    sn3, pa3, 0.0, _bcast_inner(ones65[:, 0:1], CHW),
                    ALU.max, ALU.mult)
            S = S_next

        # ---- layer 2 + readout ----
        hnode2, ex = layer(2, S)
        vo = smpool.tile([NPG, 2 * GPC], F16, tag="vo")
        nc.gpsimd.memset(vo[:], 1.0)
        nc.gpsimd.tensor_copy(vo[:, 0:2 * GPC:2], hnode2[:, 0:4 * GPC:4])
        pq = ps_ag.tile([NPG, 2 * GPC], F32, tag="pa")
        for g in range(GPC):
            nc.tensor.matmul(pq[:, 2 * g:2 * g + 2],
                             ex[:, g * NPG:(g + 1) * NPG],
                             vo[:, 2 * g:2 * g + 2], start=True, stop=True)
        rec2 = smpool.tile([NPG, GPC], F32, tag="rec2")
        nc.vector.reciprocal(rec2[:], pq[:, 1:2 * GPC:2])
        qt = smpool.tile([NPG, GPC], F16, tag="qt")
        nc.vector.scalar_tensor_tensor(qt[:], pq[:, 0:2 * GPC:2], 1.0,
                                       rec2[:], ALU.mult, ALU.mult)
        onescol = smpool.tile([NPG, 1], F16, tag="ones")
        nc.gpsimd.memset(onescol[:], 1.0)
        zps = ps_hn.tile([GPC, 1], F32, tag="ps")
        nc.tensor.matmul(zps[:], qt[:], onescol[:], start=True, stop=True)
        zout = smpool.tile([GPC, 1], F32, tag="zout")
        nc.scalar.activation(zout[:], zps[:], AF.Relu, bias=float(tail_bias))
        nc.sync.dma_start(out_d.rearrange("(g o) -> g o", o=1), zout[:])

    nc.compile()
    return nc


def kernel(**inputs):
    pre = _host_preprocess(inputs)
    nc = _build_program(pre['tail_bias'])
    in_maps = [_core_inputs(pre, c) for c in range(NC_CORES)]
    res = run_bass_kernel_spmd(nc, in_maps, list(range(NC_CORES)))
    out = np.concatenate([np.asarray(res.results[c]['out'])
                          for c in range(NC_CORES)])
    return out.reshape(B, 1).astype(np.float32)

